# revision 5
# baseline (speedup 1.0000x reference)
"""CRCDLoss Trainium2 kernel (8-core SPMD, Bass/Tile), v3.

Core strategy: replace the reference's per-(b,k) gather (~1.07 GB) with a
dense score matmul S[b, n] = v_b . mem_n over the full memory banks,
sharded along n across the 8 cores. The multiplicity counts
cnt[b, n] = #{k : idx_all[b,k] == n} are folded INTO THE EXPONENT:
one DoubleRow fp8 matmul per 512-col window computes both sides' scores
(PSUM rows 0:64 = v_s.mem2, 64:128 = v_t.mem1) and a second plain
identity matmul accumulates L[b, n] = ln(cnt)/escale_b into the same
PSUM bank, so that the activation exp(escale*(S+L)) = cnt * e directly.
The activation's free accum_out then yields the moment M1 = sum cnt*e
with no vector-engine pass at all. M2 is estimated from a stride-16
subsample of (cnt*e)^2 rescaled by the host ratio sum(cnt)/sum(cnt^2)
(valid since cnt and e are independent), entering only a tiny 2nd-order
series term. The Z normalizer is eliminated algebraically via the
2-term log series; the host combines per-core sums in float64.
Positives use host-gathered f32 rows against the normalized v.

Per-core HBM traffic: banks fused fp8 3.28MB + L fp8 1.64MB + embed
fp8 0.6MB => ~5.6MB (~16us at 358 GB/s), vs 9.8MB bf16 in v1.
"""

import sys

import numpy as np

try:
    import concourse.bass as bass  # noqa: F401
except ImportError:
    sys.path.insert(0, "/opt/trn_rl_repo")

import concourse.bacc as bacc
import concourse.bass as bass  # noqa: F811
import concourse.mybir as mybir
import concourse.tile as tile
from concourse.bass_utils import run_bass_kernel_spmd

import ml_dtypes

# ---- problem constants (hardcoded; must match the reference) ----
B = 64
D = 128
S_DIM = 1024
T_DIM = 2048
NCE_K = 16384
KP1 = NCE_K + 1          # 16385
N_DATA = 100000
NCE_T = 0.07
EPS = 1e-7
PN = 1.0 / N_DATA
CVAL = NCE_K * PN + EPS  # c = m*Pn + eps

N_CORES = 8
W = 512                  # matmul window along n (psum-bank aligned)
N_WIN = 25
R = N_WIN * W            # 12800 padded bank rows per core (12500 real)
N_PAD = N_CORES * R      # 102400 padded table rows
GRPS = [4, 4, 4, 4, 4, 4, 1]   # windows per group (4 = one PSUM quad)
M2_STRIDE = 16
BANK_SCALE = 32.0        # bank values pre-scaled for fp8e4m3
WSCALE = 16.0            # projection weights pre-scaled for fp8e4m3
LCLAMP = 240.0           # fp8e4m3 max; cnt=0 slots get L = -240

F32 = mybir.dt.float32
BF16 = mybir.dt.bfloat16
FP8 = mybir.dt.float8e4
U32 = mybir.dt.uint32

TRACE = False            # test.py can flip this for profiling runs
_CACHE = {}


def _build_program():
    nc = bacc.Bacc("TRN2", target_bir_lowering=False, debug=False,
                   num_devices=N_CORES)

    # ---- I/O ----
    wsT = nc.dram_tensor("wsT", [D, (S_DIM // D) * D], FP8,
                         kind="ExternalInput")
    wtT = nc.dram_tensor("wtT", [D, (T_DIM // D) * D], FP8,
                         kind="ExternalInput")
    fsT = nc.dram_tensor("fsT", [D, (S_DIM // D) * B], FP8,
                         kind="ExternalInput")
    ftT = nc.dram_tensor("ftT", [D, (T_DIM // D) * B], FP8,
                         kind="ExternalInput")
    bsv = nc.dram_tensor("bsv", [D, 1], F32, kind="ExternalInput")
    btv = nc.dram_tensor("btv", [D, 1], F32, kind="ExternalInput")
    # fused interleaved banks: [d, (w k j)] with k=0 -> 32*mem2T (s side),
    # k=1 -> 32*mem1T (t side)
    memf = nc.dram_tensor("memf", [D, 2 * R], FP8, kind="ExternalInput")
    lnc = nc.dram_tensor("lnc", [D, R], FP8, kind="ExternalInput")
    ident = nc.dram_tensor("ident", [D, D], FP8, kind="ExternalInput")
    pos1T = nc.dram_tensor("pos1T", [D, B], F32, kind="ExternalInput")
    pos2T = nc.dram_tensor("pos2T", [D, B], F32, kind="ExternalInput")
    rinvs = nc.dram_tensor("rinvs", [1, B], F32, kind="ExternalInput")
    rinvt = nc.dram_tensor("rinvt", [1, B], F32, kind="ExternalInput")
    escv = nc.dram_tensor("escv", [D, 1], F32, kind="ExternalInput")
    out_acc = nc.dram_tensor("out_acc", [D, 8], F32, kind="ExternalOutput")

    gpos = [0]
    for x in GRPS:
        gpos.append(gpos[-1] + x)

    with tile.TileContext(nc) as tc:
        with tc.tile_pool(name="persist", bufs=1) as pp, \
             tc.tile_pool(name="grp", bufs=2) as gp, \
             tc.tile_pool(name="psum", bufs=2, space="PSUM") as psp:

            # ---- small/latency-critical DMAs first (sync queue is FIFO):
            # embed inputs, identity, positives; then the bulk bank/L
            # streams, interleaved per group so group 0 lands early.
            # Banks on the sync (SP) HWDGE ring, L on the scalar (ACT) ring.
            def load(pool, shape, dtype, src, tag, engine=nc.sync):
                t = pool.tile(shape, dtype, tag=tag)
                engine.dma_start(out=t[:], in_=src)
                return t

            wt_s = load(pp, [D, S_DIM // D, D], FP8,
                        wsT[:].rearrange("p (c d) -> p c d", c=S_DIM // D),
                        "wt_s")
            ft_s = load(pp, [D, S_DIM // D, B], FP8,
                        fsT[:].rearrange("p (c b) -> p c b", c=S_DIM // D),
                        "ft_s")
            wt_t = load(pp, [D, T_DIM // D, D], FP8,
                        wtT[:].rearrange("p (c d) -> p c d", c=T_DIM // D),
                        "wt_t", nc.scalar)
            ft_t = load(pp, [D, T_DIM // D, B], FP8,
                        ftT[:].rearrange("p (c b) -> p c b", c=T_DIM // D),
                        "ft_t", nc.scalar)
            bs_t = load(pp, [D, 1], F32, bsv[:], "bs_t")
            bt_t = load(pp, [D, 1], F32, btv[:], "bt_t", nc.scalar)
            id_t = load(pp, [D, D], FP8, ident[:], "id_t")
            p1 = load(pp, [D, B], F32, pos1T[:], "p1", nc.scalar)
            p2 = load(pp, [D, B], F32, pos2T[:], "p2")
            ri_s = load(pp, [1, B], F32, rinvs[:], "ri_s")
            ri_t = load(pp, [1, B], F32, rinvt[:], "ri_t", nc.scalar)
            escale = load(pp, [D, 1], F32, escv[:], "escale")

            bank_t = pp.tile([D, N_WIN, 2, W], FP8, tag="bank")
            lnc_t = pp.tile([D, R], FP8, tag="lnc")
            memf_v = memf[:].rearrange("p (w k j) -> p w k j", w=N_WIN, k=2)
            for g in range(len(GRPS)):
                gsl = slice(gpos[g], gpos[g + 1])
                csl = slice(gpos[g] * W, gpos[g + 1] * W)
                nc.sync.dma_start(out=bank_t[:, gsl], in_=memf_v[:, gsl])
                nc.scalar.dma_start(out=lnc_t[:, csl], in_=lnc[:, csl])

            # ---- constants ----
            ones_col = pp.tile([D, 1], F32)      # [128, 1] of 1.0
            nc.vector.memset(ones_col[:], 1.0)
            ones_row = pp.tile([1, D], F32)      # [1, 128] of 1.0
            nc.vector.memset(ones_row[:], 1.0)

            # trigger the Exp table load early, overlapped with the DMAs
            dummy = pp.tile([1, 1], BF16, tag="dummy")
            nc.scalar.activation(out=dummy[:], in_=ones_row[:, 0:1],
                                 func=mybir.ActivationFunctionType.Exp)

            # ---- PE warm-up: dummy matmuls ramp the HAM clock ----
            wz_l = pp.tile([D, D], BF16, tag="wz_l")
            wz_r = pp.tile([D, W], BF16, tag="wz_r")
            nc.vector.memset(wz_l[:], 0.0)
            nc.vector.memset(wz_r[:], 0.0)
            wz_p = psp.tile([D, W], F32, tag="q", name="wz_p",
                            padded_shape=[D, 4 * W])
            for _wu in range(10):
                nc.tensor.matmul(out=wz_p[:], lhsT=wz_l[:], rhs=wz_r[:],
                                 start=True, stop=True)

            # ---- embed: vraw.T = (W f).T + b -> [D, B]; the 1/||v||
            # normalizer comes in precomputed (rinv/escale inputs) ----
            def embed(wt, ft, bt_, rinv, n_chunks, tag):
                vps = psp.tile([D, B], F32, tag="q", name=f"vps_{tag}",
                               padded_shape=[D, 4 * W])
                for c in range(n_chunks):
                    nc.tensor.matmul(out=vps[:], lhsT=wt[:, c, :],
                                     rhs=ft[:, c, :],
                                     start=(c == 0), stop=(c == n_chunks - 1))
                vraw = pp.tile([D, B], F32, tag=f"vraw_{tag}")
                nc.vector.tensor_scalar(out=vraw[:], in0=vps[:],
                                        scalar1=float(1.0 / WSCALE),
                                        scalar2=bt_[:, 0:1],
                                        op0=mybir.AluOpType.mult,
                                        op1=mybir.AluOpType.add)
                # vT = vraw * broadcast(rinv) (for the positives)
                rb = psp.tile([D, B], F32, tag="q", name=f"rb_{tag}",
                              padded_shape=[D, 4 * W])
                nc.tensor.matmul(out=rb[:], lhsT=ones_row[:], rhs=rinv[:],
                                 start=True, stop=True)
                vT = pp.tile([D, B], F32, tag=f"vT_{tag}")
                nc.vector.tensor_tensor(out=vT[:], in0=vraw[:], in1=rb[:],
                                        op=mybir.AluOpType.mult)
                return vT, vraw

            vTs, vraw_s = embed(wt_s, ft_s, bs_t, ri_s, S_DIM // D, "s")
            vTt, vraw_t = embed(wt_t, ft_t, bt_t, ri_t, T_DIM // D, "t")

            # fused DoubleRow weights [128, (k m)] fp8:
            # k=0 slot: cols 0:64 = vraw_s, else 0; k=1: cols 64:128 = vraw_t
            wfused = pp.tile([D, 2 * D], FP8, tag="wfused")
            nc.vector.memset(wfused[:], 0.0)
            nc.vector.tensor_copy(out=wfused[:, 0:B], in_=vraw_s[:])
            nc.vector.tensor_copy(out=wfused[:, D + B:2 * D], in_=vraw_t[:])
            wf3 = wfused[:].rearrange("p (k m) -> p k m", k=2)

            # ---- positives: pacc_s[p] = sum_b pos2T * vTs (etc.) ----
            pscr = pp.tile([D, B], F32, tag="pscr")
            pscr2 = pp.tile([D, B], F32, tag="pscr2")
            pacc_s = pp.tile([D, 1], F32, tag="pacc_s")
            pacc_t = pp.tile([D, 1], F32, tag="pacc_t")
            nc.vector.scalar_tensor_tensor(
                out=pscr[:], in0=p2[:], scalar=1.0, in1=vTs[:],
                op0=mybir.AluOpType.mult, op1=mybir.AluOpType.mult,
                accum_out=pacc_s[:])
            nc.vector.scalar_tensor_tensor(
                out=pscr2[:], in0=p1[:], scalar=1.0, in1=vTt[:],
                op0=mybir.AluOpType.mult, op1=mybir.AluOpType.mult,
                accum_out=pacc_t[:])

            # ---- moment accumulators ----
            macc = [pp.tile([D, 1], F32, tag=f"macc{m}", name=f"macc{m}")
                    for m in range(2)]
            for m in range(2):
                nc.vector.memset(macc[m][:], 0.0)

            # ---- main loop: per window one DoubleRow matmul (scores for
            # both sides) + one identity matmul adding L into the same
            # bank; per quad one exp with fused M1 accumulation ----
            for g, ng in enumerate(GRPS):
                GWg = ng * W
                quad = psp.tile([D, GWg], F32, tag="q", name=f"quad_{g}",
                                padded_shape=[D, 4 * W])
                for j in range(ng):
                    w = gpos[g] + j
                    jsl = slice(j * W, (j + 1) * W)
                    nc.tensor.matmul(out=quad[:, jsl],
                                     lhsT=wf3, rhs=bank_t[:, w],
                                     start=True, stop=False,
                                     perf_mode=mybir.MatmulPerfMode.DoubleRow)
                    nc.tensor.matmul(out=quad[:, jsl], lhsT=id_t[:],
                                     rhs=lnc_t[:, w * W:(w + 1) * W],
                                     start=False, stop=True)
                eg = gp.tile([D, GWg], BF16, tag="e_g", name=f"eg_{g}",
                             padded_shape=[D, 4 * W])
                acc1 = gp.tile([D, 1], F32, tag="acc1", name=f"acc1_{g}")
                nc.scalar.activation(out=eg[:], in_=quad[:],
                                     func=mybir.ActivationFunctionType.Exp,
                                     scale=escale[:, 0:1],
                                     accum_out=acc1[:])
                # M2' = sum (cnt*e)^2 on a stride-16 subsample
                u2 = gp.tile([D, GWg // M2_STRIDE], BF16, tag="u2",
                             name=f"u2_{g}",
                             padded_shape=[D, 4 * W // M2_STRIDE])
                acc2 = gp.tile([D, 1], F32, tag="acc2", name=f"acc2_{g}")
                nc.vector.scalar_tensor_tensor(
                    out=u2[:], in0=eg[:, 0:GWg:M2_STRIDE], scalar=1.0,
                    in1=eg[:, 0:GWg:M2_STRIDE],
                    op0=mybir.AluOpType.mult, op1=mybir.AluOpType.mult,
                    accum_out=acc2[:])
                nc.vector.tensor_tensor(out=macc[0][:], in0=macc[0][:],
                                        in1=acc1[:], op=mybir.AluOpType.add)
                nc.vector.tensor_tensor(out=macc[1][:], in0=macc[1][:],
                                        in1=acc2[:], op=mybir.AluOpType.add)

            # ---- pack outputs ----
            ot = pp.tile([D, 8], F32)
            nc.vector.memset(ot[:], 0.0)
            for m in range(2):
                nc.vector.tensor_copy(out=ot[:, m:m + 1], in_=macc[m][:])
            nc.vector.tensor_copy(out=ot[:, 3:4], in_=pacc_s[:])
            nc.vector.tensor_copy(out=ot[:, 4:5], in_=pacc_t[:])
            nc.sync.dma_start(out=out_acc[:], in_=ot[:])

    nc.finalize()
    return nc


def _prepare_in_maps(f_s, f_t, idx, contrast_idx, Ws, bs, Wt, bt,
                     memory_v1, memory_v2):
    f_s = np.asarray(f_s, dtype=np.float32)
    f_t = np.asarray(f_t, dtype=np.float32)
    Ws = np.asarray(Ws, dtype=np.float32)
    Wt = np.asarray(Wt, dtype=np.float32)
    bs = np.asarray(bs, dtype=np.float32)
    bt = np.asarray(bt, dtype=np.float32)
    memory_v1 = np.asarray(memory_v1, dtype=np.float32)
    memory_v2 = np.asarray(memory_v2, dtype=np.float32)
    idx = np.asarray(idx).astype(np.int64)
    contrast_idx = np.asarray(contrast_idx).astype(np.int64)

    fp8 = ml_dtypes.float8_e4m3fn

    # ---- index prep (sharding metadata): multiplicity counts ----
    idx_all = np.concatenate([idx[:, None], contrast_idx[:, 1:]], axis=1)
    counts = np.zeros((B, N_DATA), dtype=np.float32)
    brow = np.repeat(np.arange(B), KP1)
    np.add.at(counts, (brow, idx_all.ravel()), 1.0)
    csum = float(counts.sum(dtype=np.float64))
    c2sum = float((counts.astype(np.float64) ** 2).sum())
    _CACHE["m2_ratio"] = csum / c2sum

    # ---- replicated small tensors (weights/features scaled for fp8) ----
    def arrange(mT, cols, scale):
        n_chunks = mT.shape[0] // D
        a = mT.reshape(n_chunks, D, cols).transpose(1, 0, 2).reshape(D, -1)
        return np.ascontiguousarray((a * scale).astype(fp8))

    wsT = arrange(Ws.T, D, WSCALE)
    wtT = arrange(Wt.T, D, WSCALE)
    fsT = arrange(f_s.T, B, 1.0)
    ftT = arrange(f_t.T, B, 1.0)
    bsv = bs.reshape(D, 1).astype(np.float32)
    btv = bt.reshape(D, 1).astype(np.float32)
    pos1T = np.ascontiguousarray(memory_v1[idx].T)
    pos2T = np.ascontiguousarray(memory_v2[idx].T)
    ident = np.eye(D, dtype=np.float32).astype(fp8)

    # host mirror of the device embed (same fp8-quantized operands the PE
    # sees, so rinv matches the device vraw to fp32-accumulation noise)
    def host_rinv(wq, fq, b):
        vraw = (fq.astype(np.float32) @ wq.astype(np.float32).T) / WSCALE + b
        return (1.0 / np.sqrt((vraw ** 2).sum(axis=1))).astype(np.float32)

    rinv_s = host_rinv((Ws * WSCALE).astype(fp8), f_s.astype(fp8), bs)
    rinv_t = host_rinv((Wt * WSCALE).astype(fp8), f_t.astype(fp8), bt)
    esc_s = rinv_s.astype(np.float64) / (NCE_T * BANK_SCALE)
    esc_t = rinv_t.astype(np.float64) / (NCE_T * BANK_SCALE)
    escv = np.concatenate([esc_s, esc_t]).astype(np.float32).reshape(D, 1)

    # L[row, n] = ln(cnt)/escale_row; cnt=0 -> -LCLAMP
    with np.errstate(divide="ignore"):
        lncnt = np.log(counts)          # [B, N_DATA], -inf where cnt=0
    L_s = lncnt / esc_s[:, None]
    L_t = lncnt / esc_t[:, None]
    L = np.concatenate([L_s, L_t], axis=0)      # [128, N_DATA]
    L = np.clip(L, -LCLAMP, LCLAMP)
    L8 = np.full((D, N_PAD), -LCLAMP, dtype=np.float32)
    L8[:, :N_DATA] = L
    L8 = L8.astype(fp8)

    # pad the n dimension to N_PAD (zero rows: L=-240 kills them)
    def pad_cols(a):
        out = np.zeros((a.shape[0], N_PAD), dtype=fp8)
        out[:, :N_DATA] = a
        return out

    m1s = pad_cols((memory_v1.T * BANK_SCALE).astype(fp8))
    m2s = pad_cols((memory_v2.T * BANK_SCALE).astype(fp8))

    in_maps = []
    for c in range(N_CORES):
        sl = slice(c * R, (c + 1) * R)
        # fused interleaved banks [D, N_WIN, 2, W]: k=0 -> mem2 (s side),
        # k=1 -> mem1 (t side)
        mf = np.empty((D, N_WIN, 2, W), dtype=fp8)
        mf[:, :, 0, :] = m2s[:, sl].reshape(D, N_WIN, W)
        mf[:, :, 1, :] = m1s[:, sl].reshape(D, N_WIN, W)
        in_maps.append({
            "wsT": wsT, "wtT": wtT, "fsT": fsT, "ftT": ftT,
            "bsv": bsv, "btv": btv,
            "memf": np.ascontiguousarray(mf.reshape(D, 2 * R)),
            "lnc": np.ascontiguousarray(L8[:, sl]),
            "ident": ident,
            "pos1T": pos1T, "pos2T": pos2T,
            "rinvs": rinv_s.reshape(1, B), "rinvt": rinv_t.reshape(1, B),
            "escv": escv,
        })
    return in_maps


def _combine(out_accs):
    """out_accs: per-core [128, 8] float arrays -> scalar loss (float32)."""
    outs = [np.asarray(o).astype(np.float64) for o in out_accs]
    m2_ratio = _CACHE["m2_ratio"]

    def side_loss(half, possum):
        # M1 = sum cnt*e ; M2 ~= ratio * sum (cnt*e)^2 (stride subsample)
        M = [sum(o[half, m].sum() for o in outs) for m in range(2)]
        M[1] *= float(M2_STRIDE) * m2_ratio
        Z = M[0] / (B * KP1) * N_DATA
        cz = CVAL * Z
        # sum cnt*ln(x+c) = B*KP1*ln(c) + sum_m (-1)^(m+1) M_m/(m cz^m)
        series = sum((-1.0) ** m * M[m] / ((m + 1) * cz ** (m + 1))
                     for m in range(2))
        sum_ln_xc = B * KP1 * np.log(CVAL) + series
        neg_b_loss = (possum / NCE_T - B * np.log(Z)
                      + B * NCE_K * np.log(NCE_K * PN) - sum_ln_xc)
        return -neg_b_loss / B

    s_loss = side_loss(slice(0, B), outs[0][:, 3].sum())
    t_loss = side_loss(slice(B, D), outs[0][:, 4].sum())
    return np.float32(s_loss + t_loss)


def kernel(f_s, f_t, idx, contrast_idx, Ws, bs, Wt, bt, memory_v1, memory_v2):
    in_maps = _prepare_in_maps(f_s, f_t, idx, contrast_idx, Ws, bs, Wt, bt,
                               memory_v1, memory_v2)
    if "nc" not in _CACHE:
        _CACHE["nc"] = _build_program()
    nc = _CACHE["nc"]
    res = run_bass_kernel_spmd(nc, in_maps, list(range(N_CORES)), trace=TRACE)
    _CACHE["last_results"] = res
    return kernel_combine_results(res)


def kernel_combine_results(res):
    return _combine([res.results[c]["out_acc"] for c in range(N_CORES)])
